# revision 2
# baseline (speedup 1.0000x reference)
"""CenterLoss Trainium2 kernel.

loss = mean_i clip(||features_i - centers[target_i]||^2, 1e-12, 1e12)
       + (NUM_CLASSES-1) * 1e-12        # the clipped zeros of the masked distmat

The reference builds the full [8192, 2048] distance matrix and masks out
everything but the target column; only the per-row target distance matters,
so the kernel is a gather + fused (f-c)^2-reduce:

  - data-parallel over the batch: 1024 rows per core on 8 cores
  - centers stay in HBM; rows are gathered on-device with an indirect DMA
    (SWDGE) keyed by the int32 target indices
  - DVE computes diff = f - c, ACT computes Square(diff) with a fused
    per-partition accumulate; per-chunk partials land in a [128, NCH] tile
  - the 8 per-core [128, NCH] partial tiles are summed on the host (the
    "all-reduce" of the scalar loss)

Layout per core: features shard [1024, 512] viewed as [128, 4096] (partition
p holds rows 8p..8p+7 contiguously); idx[p, s] = target[8p + s] so the
indirect gather writes centers[target] in exactly the feature layout.
"""

import numpy as np

import concourse.bacc as bacc
import concourse.bass as bass
import concourse.tile as tile
from concourse import mybir
from concourse.bass_utils import run_bass_kernel_spmd

N_CORES = 8
BATCH = 8192
FEAT = 512
NCLS = 2048
P = 128

ROWS = BATCH // N_CORES          # 1024 rows per core
SLOTS = ROWS // P                # 8 rows per partition
FREE = SLOTS * FEAT              # 4096 f32 per partition
NCH = 4                          # processing chunks per core
CH_SLOTS = SLOTS // NCH          # 2 rows per partition per chunk
CH_FREE = CH_SLOTS * FEAT        # 1024 f32 per partition per chunk

_CACHE: dict[str, object] = {}


def _build_nc():
    nc = bacc.Bacc("TRN2", target_bir_lowering=False, debug=False)

    feats = nc.dram_tensor("features", [P, FREE], mybir.dt.float32, kind="ExternalInput")
    centers = nc.dram_tensor("centers", [NCLS, FEAT], mybir.dt.float32, kind="ExternalInput")
    idxs = nc.dram_tensor("idxs", [P, SLOTS], mybir.dt.int32, kind="ExternalInput")
    partials = nc.dram_tensor("partials", [P, NCH], mybir.dt.float32, kind="ExternalOutput")

    with tile.TileContext(nc) as tc:
        with (
            tc.tile_pool(name="idxp", bufs=1) as idxp,
            tc.tile_pool(name="accp", bufs=1) as accp,
            tc.tile_pool(name="fp", bufs=2) as fp,
            tc.tile_pool(name="cp", bufs=2) as cp,
            tc.tile_pool(name="dp", bufs=2) as dp,
            tc.tile_pool(name="sp", bufs=2) as sp,
        ):
            idx_t = idxp.tile([P, SLOTS], mybir.dt.int32)
            nc.sync.dma_start(idx_t[:], idxs[:])

            accs = accp.tile([P, NCH], mybir.dt.float32)

            for k in range(NCH):
                f_t = fp.tile([P, CH_FREE], mybir.dt.float32)
                nc.sync.dma_start(f_t[:], feats[:, k * CH_FREE:(k + 1) * CH_FREE])

                c_t = cp.tile([P, CH_FREE], mybir.dt.float32)
                nc.gpsimd.indirect_dma_start(
                    out=c_t[:],
                    out_offset=None,
                    in_=centers[:],
                    in_offset=bass.IndirectOffsetOnAxis(
                        ap=idx_t[:, k * CH_SLOTS:(k + 1) * CH_SLOTS],
                        axis=0,
                    ),
                )

                d_t = dp.tile([P, CH_FREE], mybir.dt.float32)
                nc.vector.tensor_tensor(
                    out=d_t[:], in0=f_t[:], in1=c_t[:], op=mybir.AluOpType.subtract
                )

                # out = (d * 1.0) * d = d^2, accum_out = sum(out) per partition.
                # Done on DVE (exact f32 ALU) — ACT's Square is LUT-approximated
                # and costs ~1e-3 relative error on the summed loss.
                s_t = sp.tile([P, CH_FREE], mybir.dt.float32)
                nc.vector.scalar_tensor_tensor(
                    out=s_t[:],
                    in0=d_t[:],
                    scalar=1.0,
                    in1=d_t[:],
                    op0=mybir.AluOpType.mult,
                    op1=mybir.AluOpType.mult,
                    accum_out=accs[:, k:k + 1],
                )

            nc.sync.dma_start(partials[:], accs[:])

    nc.compile()
    return nc


def _get_nc():
    if "nc" not in _CACHE:
        _CACHE["nc"] = _build_nc()
    return _CACHE["nc"]


def kernel(features: np.ndarray, centers: np.ndarray, target: np.ndarray) -> np.ndarray:
    nc = _get_nc()

    feats = np.ascontiguousarray(features, dtype=np.float32).reshape(N_CORES, P, FREE)
    cent = np.ascontiguousarray(centers, dtype=np.float32)
    idx = np.ascontiguousarray(target.astype(np.int32)).reshape(N_CORES, P, SLOTS)

    in_maps = [
        {"features": feats[i], "centers": cent, "idxs": idx[i]}
        for i in range(N_CORES)
    ]
    res = run_bass_kernel_spmd(nc, in_maps, core_ids=list(range(N_CORES)))

    total = 0.0
    for r in res.results:
        total += float(r["partials"].astype(np.float64).sum())
    loss = total / BATCH + (NCLS - 1) * 1e-12
    return np.asarray(loss, dtype=np.float32)
